# revision 19
# baseline (speedup 1.0000x reference)
"""Multi-head causal attention with RoPE on 8 TRN2 NeuronCores.

Problem: B=2, S=2048, D=1024, H=16 heads, DH=64, fp32 in/out, causal, RoPE.

Sharding (hardcoded): core c in 0..7 handles batch b = c//4 and head group
g = c%4 (heads 4g..4g+3, channels 256g..256g+256). Each core computes its
4 heads end-to-end (QKV projections, RoPE, attention, its slice of the
output projection); the host sums the 4 partial output projections per
batch in fp32. RoPE tables replicated.

Device algorithm (per core), fp16 operands with fp32 PSUM accumulation:
  - load x^T in prepacked [128, st, e, 512] chunks; project q^T,k^T per
    head pair [128, 2048] (channels on partitions) and v in natural layout
    with riding ones columns (v_ext) for softmax denominators.
  - q/k channel->partition order is host-permuted (per 64-channel head
    half: [0:16, 32:48, 16:32, 48:64]) so the RoPE half-rotation is an
    intra-quadrant lane swap done by one DVE stream_shuffle (no DMAs);
    then 3 DVE fp16 ops (mul/mul/add) with host-permuted cos /
    sign-folded-sin tables. Channel order cancels inside q.k dot products.
  - attention per (q-tile, head-pair) in transposed-score space S^T[k,q]:
    both heads' score tiles go to one 2-bank PSUM tile so a single ACT
    instruction computes exp for the pair (scale=1/8 fused, no max
    subtraction -- scores are O(5), safe); causal: k-tiles above the
    diagonal skipped, diagonal blocks narrowed, true-diagonal 128x128 gets
    a triangle mask multiply on GPSIMD (both heads in one op).
  - AV per pair: each head accumulates [v|ones] into a [65, 512] PSUM tile
    (denominator rides in row 64). Normalize: reciprocal on the den rows,
    K=1 matmuls broadcast the recips across 64 partitions, 2 DVE
    multiplies; the odd head's tile is stacked into attnP rows 64..127 by
    a small SBUF->SBUF DMA so the output projection can contract the pair
    with K=128 matmuls (all 4 heads in 2 accumulating matmuls per tile).
  - output projection contracts the pair-stacked attnP (K=128, two
    accumulating matmuls per tile = all 4 heads), fp16 result DMA'd per
    512-row chunk; host upcasts and sums.
"""
import numpy as np

B, S, D, H = 2, 2048, 1024, 16
DH = 64
NCORES = 8
P = 128
QT = 512                  # q tile (free dim)
NQT = S // QT             # 4
NKT = S // P              # 16 k tiles
NE = D // P               # 8 contraction chunks
HPC = 4                   # heads per core
C = HPC * DH              # 256 channels per core

# channel->partition permutation per 64-channel head half (makes rotate-half
# an intra-quadrant 16<->16 lane swap), and the matching DVE shuffle mask
PERM64 = np.r_[0:16, 32:48, 16:32, 48:64]
SHUF = list(range(16, 32)) + list(range(16))

_cache = {}


def _build():
    import concourse.mybir as mybir
    import concourse.tile as tile
    from concourse import bacc

    F16 = mybir.dt.float16
    F32 = mybir.dt.float32
    MUL = mybir.AluOpType.mult
    ADD = mybir.AluOpType.add
    EXP = mybir.ActivationFunctionType.Exp

    nc = bacc.Bacc(trn_type="TRN2", target_bir_lowering=False, debug=False,
                   enable_asserts=False)
    xTp = nc.dram_tensor("xTp", [P, NQT, NE, QT], F16, kind="ExternalInput").ap()
    wq_t = nc.dram_tensor("wq_t", [P, NE, C], F16, kind="ExternalInput").ap()
    wk_t = nc.dram_tensor("wk_t", [P, NE, C], F16, kind="ExternalInput").ap()
    wv_t = nc.dram_tensor("wv_t", [P, NE, C], F16, kind="ExternalInput").ap()
    wo_p = nc.dram_tensor("wo_p", [P, 2, D], F16, kind="ExternalInput").ap()
    cos2 = nc.dram_tensor("cos2", [P, S], F16, kind="ExternalInput").ap()
    sin2 = nc.dram_tensor("sin2", [P, S], F16, kind="ExternalInput").ap()
    mask2 = nc.dram_tensor("mask2", [P, 2, P], F16, kind="ExternalInput").ap()
    onesd = nc.dram_tensor("onesd", [P, DH], F16, kind="ExternalInput").ap()
    y = nc.dram_tensor("y", [S, D], F16, kind="ExternalOutput").ap()

    with tile.TileContext(nc) as tc:
        with tc.tile_pool(name="keep", bufs=1) as keep, \
             tc.tile_pool(name="ph1", bufs=2) as ph1, \
             tc.tile_pool(name="swp", bufs=2) as swp, \
             tc.tile_pool(name="ptp", bufs=6) as ptp, \
             tc.tile_pool(name="normp", bufs=2) as normp, \
             tc.tile_pool(name="work", bufs=2) as work, \
             tc.tile_pool(name="psA", bufs=2, space="PSUM") as psA, \
             tc.tile_pool(name="psS", bufs=2, space="PSUM") as psS, \
             tc.tile_pool(name="psO", bufs=1, space="PSUM") as psO:

            # ---------------- persistent tiles ----------------
            qk_pair = {(w, pr): keep.tile([P, S], F16, tag=f"{w}{pr}",
                                          name=f"{w}{pr}")
                       for w in ("q", "k") for pr in range(2)}
            v_ext = keep.tile([P, NKT, HPC * (DH + 1)], F16, tag="vext")
            attnP = [keep.tile([P, S], F16, tag=f"attnP{pr}",
                               name=f"attnP{pr}") for pr in range(2)]
            wo_sb = keep.tile([P, 2, D], F16, tag="wo")
            cos_sb = keep.tile([P, S], F16, tag="cos")
            sin_sb = keep.tile([P, S], F16, tag="sin")
            wq_sb = keep.tile([P, NE, C], F16, tag="wq")
            wk_sb = keep.tile([P, NE, C], F16, tag="wk")
            wv_sb = keep.tile([P, NE, C], F16, tag="wv")
            mask_sb = keep.tile([P, 2, P], F16, tag="mask")
            ones_sb = keep.tile([DH + 1, DH], F16, tag="ones")

            vx = v_ext.rearrange("p t (h x) -> p t h x", h=HPC)

            # ---------------- input loads ----------------
            nc.sync.dma_start(wq_sb, wq_t)
            nc.sync.dma_start(wk_sb, wk_t)
            nc.sync.dma_start(wv_sb, wv_t)
            nc.sync.dma_start(cos_sb, cos2)
            nc.sync.dma_start(sin_sb, sin2)
            nc.sync.dma_start(mask_sb, mask2)
            nc.sync.dma_start(ones_sb, onesd[:DH + 1])
            nc.sync.dma_start(
                vx[:, :, :, DH:],
                onesd.rearrange("p (t h) -> p t h", t=NKT)[:, :, :, None])
            nc.sync.dma_start(wo_sb, wo_p)

            w_of = {"q": wq_sb, "k": wk_sb}

            def phase1(st):
                """QKV projections + RoPE for s-tile st."""
                xt = ph1.tile([P, NE, QT], F16, tag="xt")
                nc.sync.dma_start(xt, xTp[:, st])
                sl = slice(st * QT, (st + 1) * QT)
                for which in ("q", "k"):
                    for pr in range(2):
                        ps = psA.tile([P, QT], F32, tag="ps", name="ps")
                        for e in range(NE):
                            nc.tensor.matmul(
                                ps,
                                lhsT=w_of[which][:, e, pr * P:(pr + 1) * P],
                                rhs=xt[:, e],
                                start=(e == 0), stop=(e == NE - 1))
                        raw = qk_pair[(which, pr)][:, sl]
                        nc.vector.tensor_copy(raw, ps)
                        sw = swp.tile([P, QT], F16, tag="sw")
                        nc.vector.stream_shuffle(sw, raw, SHUF)
                        nc.vector.tensor_tensor(sw, sw, sin_sb[:, sl], MUL)
                        nc.vector.tensor_tensor(raw, raw, cos_sb[:, sl], MUL)
                        nc.vector.tensor_tensor(raw, raw, sw, ADD)
                for sb in range(4):
                    kt = st * 4 + sb
                    pv = psA.tile([P, QT], F32, tag="ps", name="pv")[:, :C]
                    for e in range(NE):
                        nc.tensor.matmul(
                            pv,
                            lhsT=xt[:, e, sb * P:(sb + 1) * P],
                            rhs=wv_sb[:, e],
                            start=(e == 0), stop=(e == NE - 1))
                    nc.vector.tensor_copy(
                        vx[:, kt, :, :DH],
                        pv.rearrange("p (h x) -> p h x", h=HPC))

            def attention(qt):
                nkt = 4 * qt + 4
                sl = slice(qt * QT, (qt + 1) * QT)
                for pr in range(2):
                    poA = psO.tile([DH + 1, QT], F32, tag="poA")
                    poB = psO.tile([DH + 1, QT], F32, tag="poB")
                    qh = [qk_pair[("q", pr)][hi * DH:(hi + 1) * DH]
                          for hi in range(2)]
                    kh = [qk_pair[("k", pr)][hi * DH:(hi + 1) * DH]
                          for hi in range(2)]
                    for kt in range(nkt):
                        j = kt - 4 * qt   # >= 0 on diagonal blocks
                        lo = max(j, 0) * P
                        ps = psS.tile([P, 2, QT], F32, tag="ps", name="pss")
                        for hi in range(2):
                            nc.tensor.matmul(
                                ps[:, hi, lo:],
                                lhsT=kh[hi][:, kt * P:(kt + 1) * P],
                                rhs=qh[hi][:, qt * QT + lo:(qt + 1) * QT])
                        pt = ptp.tile([P, 2, QT], F16, tag="pt")
                        nc.scalar.activation(pt[:, :, lo:], ps[:, :, lo:],
                                             EXP, scale=0.125)
                        if j >= 0:
                            nc.gpsimd.tensor_tensor(
                                pt[:, :, lo:lo + P], pt[:, :, lo:lo + P],
                                mask_sb, MUL)
                        nc.tensor.matmul(poA[:, lo:],
                                         lhsT=vx[:, kt, 2 * pr],
                                         rhs=pt[:, 0, lo:],
                                         start=(kt == 0), stop=(kt == nkt - 1))
                        nc.tensor.matmul(poB[:, lo:],
                                         lhsT=vx[:, kt, 2 * pr + 1],
                                         rhs=pt[:, 1, lo:],
                                         start=(kt == 0), stop=(kt == nkt - 1))
                    den2 = normp.tile([DH + 1, 2, QT], F16, tag="den2")
                    with nc.allow_low_precision(reason="softmax denom recip"):
                        nc.vector.reciprocal(den2[DH:, 0], poA[DH:])
                        nc.vector.reciprocal(den2[DH:, 1], poB[DH:])
                    bcA = psA.tile([P, QT], F32, tag="ps", name="bcA")[:DH]
                    bcB = psA.tile([P, QT], F32, tag="ps", name="bcB")[:DH]
                    nc.tensor.matmul(bcA, lhsT=ones_sb[DH:], rhs=den2[DH:, 0])
                    nc.tensor.matmul(bcB, lhsT=ones_sb[DH:], rhs=den2[DH:, 1])
                    bcs = normp.tile([DH, 2, QT], F16, tag="bcs")
                    nc.scalar.copy(bcs[:, 0], bcA)
                    nc.scalar.copy(bcs[:, 1], bcB)
                    atmp = swp.tile([DH, QT], F16, tag="atmp")
                    nc.vector.tensor_tensor(attnP[pr][:DH, sl],
                                            poA[:DH], bcs[:, 0], MUL)
                    nc.vector.tensor_tensor(atmp, poB[:DH], bcs[:, 1], MUL)
                    nc.sync.dma_start(attnP[pr][DH:, sl], atmp)

            def proj(qt):
                """Output projection for the 512 s-rows of q-tile qt."""
                ysb = work.tile([P, 4, D], F16, tag="ysb")
                for sb in range(4):
                    sc = qt * 4 + sb
                    for et in range(2):
                        psy = psA.tile([P, QT], F32, tag="ps", name="psy")
                        for pr in range(2):
                            nc.tensor.matmul(
                                psy,
                                lhsT=attnP[pr][:, sc * P:(sc + 1) * P],
                                rhs=wo_sb[:, pr, et * QT:(et + 1) * QT],
                                start=(pr == 0), stop=(pr == 1))
                        dst = ysb[:, sb, et * QT:(et + 1) * QT]
                        if (sb + et) % 2 == 0:
                            nc.vector.tensor_copy(dst, psy)
                        else:
                            nc.scalar.copy(dst, psy)
                nc.sync.dma_start(
                    y[qt * QT:(qt + 1) * QT].rearrange("(c p) e -> p c e",
                                                       p=P), ysb)

            phase1(0)
            phase1(1)
            attention(0)
            proj(0)
            phase1(2)
            attention(1)
            proj(1)
            phase1(3)
            attention(2)
            proj(2)
            attention(3)
            proj(3)
    nc.compile()
    return nc


def _get_nc():
    if "nc" not in _cache:
        _cache["nc"] = _build()
    return _cache["nc"]


def _host_inputs(x, Wq, Wk, Wv, Wo, cos, sin):
    """Build the 8 per-core input dicts (fp16, prepacked layouts)."""
    f16 = np.float16
    ordH = (np.arange(HPC)[:, None] * DH + PERM64[None, :]).reshape(-1)  # [256]

    cosT = np.ascontiguousarray(cos.T).astype(np.float32)     # [DH, S]
    sinT = np.ascontiguousarray(sin.T).astype(np.float32)
    sinS = np.concatenate([-sinT[:DH // 2], sinT[DH // 2:]], axis=0)
    cos2 = np.tile(cosT[PERM64], (2, 1)).astype(f16)          # [128, S]
    sin2 = np.tile(sinS[PERM64], (2, 1)).astype(f16)
    mask1 = (np.arange(P)[:, None] <= np.arange(P)[None, :])
    mask2 = np.stack([mask1, mask1], axis=1).astype(f16)      # [128, 2, 128]
    onesd = np.ones((P, DH), f16)

    in_maps = []
    for c in range(NCORES):
        b, g = divmod(c, 4)
        cs = slice(C * g, C * g + C)
        xb = np.asarray(x[b], np.float32)                     # [S, D]
        xTp = xb.reshape(NQT, QT, NE, P).transpose(3, 0, 2, 1)
        wq_o = np.asarray(Wq, np.float32)[cs][ordH]           # [256, D]
        wk_o = np.asarray(Wk, np.float32)[cs][ordH]
        wv_o = np.asarray(Wv, np.float32)[cs]
        wo_o = np.asarray(Wo, np.float32).T[cs]               # [256, D]
        in_maps.append({
            "xTp": np.ascontiguousarray(xTp).astype(f16),
            "wq_t": np.ascontiguousarray(
                wq_o.T.reshape(NE, P, C).transpose(1, 0, 2)).astype(f16),
            "wk_t": np.ascontiguousarray(
                wk_o.T.reshape(NE, P, C).transpose(1, 0, 2)).astype(f16),
            "wv_t": np.ascontiguousarray(
                wv_o.T.reshape(NE, P, C).transpose(1, 0, 2)).astype(f16),
            "wo_p": np.ascontiguousarray(
                wo_o.reshape(2, P, D).transpose(1, 0, 2)).astype(f16),
            "cos2": cos2, "sin2": sin2, "mask2": mask2, "onesd": onesd,
        })
    return in_maps


def run(x, Wq, Wk, Wv, Wo, cos, sin, mask=None, trace=False, **trace_kw):
    import os
    import time
    if not trace:
        # The axon NTFF-profile hook is not installed in all containers;
        # make sure an inherited BASS_TRACE=1 can't send us down that path.
        os.environ.setdefault("BASS_NEVER_TRACE", "1")
    from concourse.bass_utils import run_bass_kernel_spmd
    nc = _get_nc()
    in_maps = _host_inputs(np.asarray(x), np.asarray(Wq), np.asarray(Wk),
                           np.asarray(Wv), np.asarray(Wo), np.asarray(cos),
                           np.asarray(sin))
    try:
        res = run_bass_kernel_spmd(nc, in_maps, core_ids=list(range(NCORES)),
                                   trace=trace, **trace_kw)
    except Exception:
        # one retry for transient device states (e.g. NRT_EXEC_UNIT errors)
        time.sleep(15)
        res = run_bass_kernel_spmd(nc, in_maps, core_ids=list(range(NCORES)),
                                   trace=trace, **trace_kw)
    parts = [r["y"].astype(np.float32) for r in res.results]
    out = np.stack([parts[0] + parts[1] + parts[2] + parts[3],
                    parts[4] + parts[5] + parts[6] + parts[7]])
    return out.astype(np.float32), res


def kernel(x, Wq, Wk, Wv, Wo, cos, sin, mask=None, **_):
    out, _res = run(x, Wq, Wk, Wv, Wo, cos, sin, mask)
    return out


# revision 23
# speedup vs baseline: 1.0610x; 1.0610x over previous
"""Multi-head causal attention with RoPE on 8 TRN2 NeuronCores.

Problem: B=2, S=2048, D=1024, H=16 heads, DH=64, fp32 in/out, causal, RoPE.

Sharding (hardcoded): core c in 0..7 handles batch b = c//4 and head group
g = c%4 (heads 4g..4g+3, channels 256g..256g+256). Each core computes its
4 heads end-to-end (QKV projections, RoPE, attention, its slice of the
output projection); the host sums the 4 partial output projections per
batch in fp32. RoPE tables replicated.

Device algorithm (per core), fp16 operands with fp32 PSUM accumulation:
  - load x^T in prepacked [128, st, e, 512] chunks; project q^T,k^T per
    head pair [128, 2048] (channels on partitions) and v in natural layout
    with riding ones columns (v_ext) for softmax denominators.
  - q/k channel->partition order is host-permuted (per 64-channel head
    half: [0:16, 32:48, 16:32, 48:64]) so the RoPE half-rotation is an
    intra-quadrant lane swap done by one DVE stream_shuffle (no DMAs);
    then 3 DVE fp16 ops (mul/mul/add) with host-permuted cos /
    sign-folded-sin tables. Channel order cancels inside q.k dot products.
  - attention per (q-tile, head-pair) in transposed-score space S^T[k,q]:
    both heads' score tiles go to one 2-bank PSUM tile so a single ACT
    instruction computes exp for the pair (scale=1/8 fused, no max
    subtraction -- scores are O(5), safe); causal: k-tiles above the
    diagonal skipped, diagonal blocks narrowed, true-diagonal 128x128 gets
    a triangle mask multiply on GPSIMD (both heads in one op).
  - AV per pair: each head accumulates [v|ones] into a [65, 512] PSUM tile
    (denominator rides in row 64). Normalize: reciprocal on the den rows,
    K=1 matmuls broadcast the recips across 64 partitions, 2 DVE
    multiplies; the odd head's tile is stacked into attnP rows 64..127 by
    a small SBUF->SBUF DMA so the output projection can contract the pair
    with K=128 matmuls (all 4 heads in 2 accumulating matmuls per tile).
  - output projection contracts the pair-stacked attnP (K=128, two
    accumulating matmuls per tile = all 4 heads), fp16 result DMA'd per
    512-row chunk; host upcasts and sums.
"""
import numpy as np

B, S, D, H = 2, 2048, 1024, 16
DH = 64
NCORES = 8
P = 128
QT = 512                  # q tile (free dim)
NQT = S // QT             # 4
NKT = S // P              # 16 k tiles
NE = D // P               # 8 contraction chunks
HPC = 4                   # heads per core
C = HPC * DH              # 256 channels per core

# channel->partition permutation per 64-channel head half (makes rotate-half
# an intra-quadrant 16<->16 lane swap), and the matching DVE shuffle mask
PERM64 = np.r_[0:16, 32:48, 16:32, 48:64]
SHUF = list(range(16, 32)) + list(range(16))

_cache = {}


def _build():
    import concourse.mybir as mybir
    import concourse.tile as tile
    from concourse import bacc

    F16 = mybir.dt.float16
    F32 = mybir.dt.float32
    MUL = mybir.AluOpType.mult
    ADD = mybir.AluOpType.add
    EXP = mybir.ActivationFunctionType.Exp

    nc = bacc.Bacc(trn_type="TRN2", target_bir_lowering=False, debug=False,
                   enable_asserts=False)
    xTp = nc.dram_tensor("xTp", [P, NQT, NE, QT], F16, kind="ExternalInput").ap()
    wq_t = nc.dram_tensor("wq_t", [P, NE, C], F16, kind="ExternalInput").ap()
    wk_t = nc.dram_tensor("wk_t", [P, NE, C], F16, kind="ExternalInput").ap()
    wv_t = nc.dram_tensor("wv_t", [P, NE, C], F16, kind="ExternalInput").ap()
    wo_p = nc.dram_tensor("wo_p", [P, 2, D], F16, kind="ExternalInput").ap()
    cos2 = nc.dram_tensor("cos2", [P, S], F16, kind="ExternalInput").ap()
    sin2 = nc.dram_tensor("sin2", [P, S], F16, kind="ExternalInput").ap()
    mask2 = nc.dram_tensor("mask2", [P, 2, P], F16, kind="ExternalInput").ap()
    onesd = nc.dram_tensor("onesd", [P, DH], F16, kind="ExternalInput").ap()
    y = nc.dram_tensor("y", [S, D], F16, kind="ExternalOutput").ap()

    with tile.TileContext(nc) as tc:
        with tc.tile_pool(name="keep", bufs=1) as keep, \
             tc.tile_pool(name="ph1", bufs=2) as ph1, \
             tc.tile_pool(name="swp", bufs=2) as swp, \
             tc.tile_pool(name="ptp", bufs=6) as ptp, \
             tc.tile_pool(name="normp", bufs=2) as normp, \
             tc.tile_pool(name="work", bufs=2) as work, \
             tc.tile_pool(name="psA", bufs=2, space="PSUM") as psA, \
             tc.tile_pool(name="psS", bufs=2, space="PSUM") as psS, \
             tc.tile_pool(name="psO", bufs=1, space="PSUM") as psO:

            # ---------------- persistent tiles ----------------
            qk_pair = {(w, pr): keep.tile([P, S], F16, tag=f"{w}{pr}",
                                          name=f"{w}{pr}")
                       for w in ("q", "k") for pr in range(2)}
            v_ext = keep.tile([P, NKT, HPC * (DH + 1)], F16, tag="vext")
            attnP = [keep.tile([P, S], F16, tag=f"attnP{pr}",
                               name=f"attnP{pr}") for pr in range(2)]
            wo_sb = keep.tile([P, 2, D], F16, tag="wo")
            cos_sb = keep.tile([P, S], F16, tag="cos")
            sin_sb = keep.tile([P, S], F16, tag="sin")
            wq_sb = keep.tile([P, NE, C], F16, tag="wq")
            wk_sb = keep.tile([P, NE, C], F16, tag="wk")
            wv_sb = keep.tile([P, NE, C], F16, tag="wv")
            mask_sb = keep.tile([P, 2, P], F16, tag="mask")
            ones_sb = keep.tile([DH + 1, DH], F16, tag="ones")

            vx = v_ext.rearrange("p t (h x) -> p t h x", h=HPC)

            # ---------------- input loads ----------------
            # phase1(0) blockers first: x chunk 0 and the q/k/v weights.
            xts = {0: ph1.tile([P, NE, QT], F16, tag="xt", name="xt0")}
            nc.sync.dma_start(xts[0], xTp[:, 0])
            nc.sync.dma_start(wq_sb, wq_t)
            nc.sync.dma_start(wk_sb, wk_t)
            nc.sync.dma_start(wv_sb, wv_t)
            nc.sync.dma_start(cos_sb, cos2)
            nc.sync.dma_start(sin_sb, sin2)

            def late_loads():
                nc.sync.dma_start(mask_sb, mask2)
                nc.sync.dma_start(ones_sb, onesd[:DH + 1])
                nc.sync.dma_start(
                    vx[:, :, :, DH:],
                    onesd.rearrange("p (t h) -> p t h", t=NKT)[:, :, :, None])
                nc.sync.dma_start(wo_sb, wo_p)

            w_of = {"q": wq_sb, "k": wk_sb}

            def phase1(st):
                """QKV projections + RoPE for s-tile st."""
                if st in xts:
                    xt = xts.pop(st)
                else:
                    xt = ph1.tile([P, NE, QT], F16, tag="xt")
                    nc.sync.dma_start(xt, xTp[:, st])
                sl = slice(st * QT, (st + 1) * QT)
                for which in ("q", "k"):
                    for pr in range(2):
                        ps = psA.tile([P, QT], F32, tag="ps", name="ps")
                        for e in range(NE):
                            nc.tensor.matmul(
                                ps,
                                lhsT=w_of[which][:, e, pr * P:(pr + 1) * P],
                                rhs=xt[:, e],
                                start=(e == 0), stop=(e == NE - 1))
                        raw = qk_pair[(which, pr)][:, sl]
                        nc.vector.tensor_copy(raw, ps)
                        sw = swp.tile([P, QT], F16, tag="sw")
                        nc.vector.stream_shuffle(sw, raw, SHUF)
                        nc.vector.tensor_tensor(sw, sw, sin_sb[:, sl], MUL)
                        nc.vector.tensor_tensor(raw, raw, cos_sb[:, sl], MUL)
                        nc.vector.tensor_tensor(raw, raw, sw, ADD)
                for sb in range(4):
                    kt = st * 4 + sb
                    pv = psA.tile([P, QT], F32, tag="ps", name="pv")[:, :C]
                    for e in range(NE):
                        nc.tensor.matmul(
                            pv,
                            lhsT=xt[:, e, sb * P:(sb + 1) * P],
                            rhs=wv_sb[:, e],
                            start=(e == 0), stop=(e == NE - 1))
                    nc.vector.tensor_copy(
                        vx[:, kt, :, :DH],
                        pv.rearrange("p (h x) -> p h x", h=HPC))

            def attention(qt):
                nkt = 4 * qt + 4
                sl = slice(qt * QT, (qt + 1) * QT)
                for pr in range(2):
                    poA = psO.tile([DH + 1, QT], F32, tag="poA")
                    poB = psO.tile([DH + 1, QT], F32, tag="poB")
                    qh = [qk_pair[("q", pr)][hi * DH:(hi + 1) * DH]
                          for hi in range(2)]
                    kh = [qk_pair[("k", pr)][hi * DH:(hi + 1) * DH]
                          for hi in range(2)]
                    for kt in range(nkt):
                        j = kt - 4 * qt   # >= 0 on diagonal blocks
                        lo = max(j, 0) * P
                        ps = psS.tile([P, 2, QT], F32, tag="ps", name="pss")
                        for hi in range(2):
                            nc.tensor.matmul(
                                ps[:, hi, lo:],
                                lhsT=kh[hi][:, kt * P:(kt + 1) * P],
                                rhs=qh[hi][:, qt * QT + lo:(qt + 1) * QT])
                        pt = ptp.tile([P, 2, QT], F16, tag="pt")
                        nc.scalar.activation(pt[:, :, lo:], ps[:, :, lo:],
                                             EXP, scale=0.125)
                        if j >= 0:
                            nc.gpsimd.tensor_tensor(
                                pt[:, :, lo:lo + P], pt[:, :, lo:lo + P],
                                mask_sb, MUL)
                        nc.tensor.matmul(poA[:, lo:],
                                         lhsT=vx[:, kt, 2 * pr],
                                         rhs=pt[:, 0, lo:],
                                         start=(kt == 0), stop=(kt == nkt - 1))
                        nc.tensor.matmul(poB[:, lo:],
                                         lhsT=vx[:, kt, 2 * pr + 1],
                                         rhs=pt[:, 1, lo:],
                                         start=(kt == 0), stop=(kt == nkt - 1))
                    den2 = normp.tile([DH + 1, 2, QT], F16, tag="den2")
                    with nc.allow_low_precision(reason="softmax denom recip"):
                        nc.vector.reciprocal(den2[DH:, 0], poA[DH:])
                        nc.vector.reciprocal(den2[DH:, 1], poB[DH:])
                    bcA = psA.tile([P, QT], F32, tag="ps", name="bcA")[:DH]
                    bcB = psA.tile([P, QT], F32, tag="ps", name="bcB")[:DH]
                    nc.tensor.matmul(bcA, lhsT=ones_sb[DH:], rhs=den2[DH:, 0])
                    nc.tensor.matmul(bcB, lhsT=ones_sb[DH:], rhs=den2[DH:, 1])
                    bcs = normp.tile([DH, 2, QT], F16, tag="bcs")
                    nc.scalar.copy(bcs[:, 0], bcA)
                    nc.vector.tensor_copy(bcs[:, 1], bcB)
                    atmp = swp.tile([DH, QT], F16, tag="atmp")
                    nc.vector.tensor_tensor(attnP[pr][:DH, sl],
                                            poA[:DH], bcs[:, 0], MUL)
                    nc.vector.tensor_tensor(atmp, poB[:DH], bcs[:, 1], MUL)
                    nc.sync.dma_start(attnP[pr][DH:, sl], atmp)

            def proj(qt):
                """Output projection for the 512 s-rows of q-tile qt."""
                ysb = work.tile([P, 4, D], F16, tag="ysb")
                for sb in range(4):
                    sc = qt * 4 + sb
                    for et in range(2):
                        psy = psA.tile([P, QT], F32, tag="ps", name="psy")
                        for pr in range(2):
                            nc.tensor.matmul(
                                psy,
                                lhsT=attnP[pr][:, sc * P:(sc + 1) * P],
                                rhs=wo_sb[:, pr, et * QT:(et + 1) * QT],
                                start=(pr == 0), stop=(pr == 1))
                        dst = ysb[:, sb, et * QT:(et + 1) * QT]
                        if et == 1 and sb < 3:
                            nc.scalar.copy(dst, psy)
                        else:
                            nc.vector.tensor_copy(dst, psy)
                nc.sync.dma_start(
                    y[qt * QT:(qt + 1) * QT].rearrange("(c p) e -> p c e",
                                                       p=P), ysb)

            phase1(0)
            late_loads()
            phase1(1)
            attention(0)
            proj(0)
            phase1(2)
            attention(1)
            proj(1)
            phase1(3)
            attention(2)
            proj(2)
            attention(3)
            proj(3)
    nc.compile()
    return nc


def _get_nc():
    if "nc" not in _cache:
        _cache["nc"] = _build()
    return _cache["nc"]


def _host_inputs(x, Wq, Wk, Wv, Wo, cos, sin):
    """Build the 8 per-core input dicts (fp16, prepacked layouts)."""
    f16 = np.float16
    ordH = (np.arange(HPC)[:, None] * DH + PERM64[None, :]).reshape(-1)  # [256]

    cosT = np.ascontiguousarray(cos.T).astype(np.float32)     # [DH, S]
    sinT = np.ascontiguousarray(sin.T).astype(np.float32)
    sinS = np.concatenate([-sinT[:DH // 2], sinT[DH // 2:]], axis=0)
    cos2 = np.tile(cosT[PERM64], (2, 1)).astype(f16)          # [128, S]
    sin2 = np.tile(sinS[PERM64], (2, 1)).astype(f16)
    mask1 = (np.arange(P)[:, None] <= np.arange(P)[None, :])
    mask2 = np.stack([mask1, mask1], axis=1).astype(f16)      # [128, 2, 128]
    onesd = np.ones((P, DH), f16)

    in_maps = []
    for c in range(NCORES):
        b, g = divmod(c, 4)
        cs = slice(C * g, C * g + C)
        xb = np.asarray(x[b], np.float32)                     # [S, D]
        xTp = xb.reshape(NQT, QT, NE, P).transpose(3, 0, 2, 1)
        wq_o = np.asarray(Wq, np.float32)[cs][ordH]           # [256, D]
        wk_o = np.asarray(Wk, np.float32)[cs][ordH]
        wv_o = np.asarray(Wv, np.float32)[cs]
        wo_o = np.asarray(Wo, np.float32).T[cs]               # [256, D]
        in_maps.append({
            "xTp": np.ascontiguousarray(xTp).astype(f16),
            "wq_t": np.ascontiguousarray(
                wq_o.T.reshape(NE, P, C).transpose(1, 0, 2)).astype(f16),
            "wk_t": np.ascontiguousarray(
                wk_o.T.reshape(NE, P, C).transpose(1, 0, 2)).astype(f16),
            "wv_t": np.ascontiguousarray(
                wv_o.T.reshape(NE, P, C).transpose(1, 0, 2)).astype(f16),
            "wo_p": np.ascontiguousarray(
                wo_o.reshape(2, P, D).transpose(1, 0, 2)).astype(f16),
            "cos2": cos2, "sin2": sin2, "mask2": mask2, "onesd": onesd,
        })
    return in_maps


def run(x, Wq, Wk, Wv, Wo, cos, sin, mask=None, trace=False, **trace_kw):
    import os
    import time
    if not trace:
        # The axon NTFF-profile hook is not installed in all containers;
        # make sure an inherited BASS_TRACE=1 can't send us down that path.
        os.environ.setdefault("BASS_NEVER_TRACE", "1")
    from concourse.bass_utils import run_bass_kernel_spmd
    nc = _get_nc()
    in_maps = _host_inputs(np.asarray(x), np.asarray(Wq), np.asarray(Wk),
                           np.asarray(Wv), np.asarray(Wo), np.asarray(cos),
                           np.asarray(sin))
    try:
        res = run_bass_kernel_spmd(nc, in_maps, core_ids=list(range(NCORES)),
                                   trace=trace, **trace_kw)
    except Exception:
        # one retry for transient device states (e.g. NRT_EXEC_UNIT errors)
        time.sleep(15)
        res = run_bass_kernel_spmd(nc, in_maps, core_ids=list(range(NCORES)),
                                   trace=trace, **trace_kw)
    parts = [r["y"].astype(np.float32) for r in res.results]
    out = np.stack([parts[0] + parts[1] + parts[2] + parts[3],
                    parts[4] + parts[5] + parts[6] + parts[7]])
    return out.astype(np.float32), res


def kernel(x, Wq, Wk, Wv, Wo, cos, sin, mask=None, **_):
    out, _res = run(x, Wq, Wk, Wv, Wo, cos, sin, mask)
    return out


# revision 26
# speedup vs baseline: 1.0783x; 1.0163x over previous
"""Multi-head causal attention with RoPE on 8 TRN2 NeuronCores.

Problem: B=2, S=2048, D=1024, H=16 heads, DH=64, fp32 in/out, causal, RoPE.

Sharding (hardcoded): core c in 0..7 handles batch b = c//4 and head group
g = c%4 (heads 4g..4g+3, channels 256g..256g+256). Each core computes its
4 heads end-to-end (QKV projections, RoPE, attention, its slice of the
output projection); the host sums the 4 partial output projections per
batch in fp32. RoPE tables replicated.

Device algorithm (per core), fp16 operands with fp32 PSUM accumulation:
  - load x^T in prepacked [128, st, e, 512] chunks; project q^T,k^T per
    head pair [128, 2048] (channels on partitions) and v in natural layout
    with riding ones columns (v_ext) for softmax denominators.
  - q/k channel->partition order is host-permuted (per 64-channel head
    half: [0:16, 32:48, 16:32, 48:64]) so the RoPE half-rotation is an
    intra-quadrant lane swap done by one DVE stream_shuffle (no DMAs);
    then 3 DVE fp16 ops (mul/mul/add) with host-permuted cos /
    sign-folded-sin tables. Channel order cancels inside q.k dot products.
  - attention per (q-tile, head-pair) in transposed-score space S^T[k,q]:
    both heads' score tiles go to one 2-bank PSUM tile so a single ACT
    instruction computes exp for the pair (scale=1/8 fused, no max
    subtraction -- scores are O(5), safe); causal: k-tiles above the
    diagonal skipped, diagonal blocks narrowed, true-diagonal 128x128 gets
    a triangle mask multiply on GPSIMD (both heads in one op).
  - AV per pair: each head accumulates [v|ones] into a [65, 512] PSUM tile
    (denominator rides in row 64). Normalize: reciprocal on the den rows,
    K=1 matmuls broadcast the recips across 64 partitions, 2 DVE
    multiplies; the odd head's tile is stacked into attnP rows 64..127 by
    a small SBUF->SBUF DMA so the output projection can contract the pair
    with K=128 matmuls (all 4 heads in 2 accumulating matmuls per tile).
  - output projection contracts the pair-stacked attnP (K=128, two
    accumulating matmuls per tile = all 4 heads), fp16 result DMA'd per
    512-row chunk; host upcasts and sums.
"""
import numpy as np

B, S, D, H = 2, 2048, 1024, 16
DH = 64
NCORES = 8
P = 128
QT = 512                  # q tile (free dim)
NQT = S // QT             # 4
NKT = S // P              # 16 k tiles
NE = D // P               # 8 contraction chunks
HPC = 4                   # heads per core
C = HPC * DH              # 256 channels per core

# channel->partition permutation per 64-channel head half (makes rotate-half
# an intra-quadrant 16<->16 lane swap), and the matching DVE shuffle mask
PERM64 = np.r_[0:16, 32:48, 16:32, 48:64]
SHUF = list(range(16, 32)) + list(range(16))

_cache = {}


def _build():
    import concourse.mybir as mybir
    import concourse.tile as tile
    from concourse import bacc

    F16 = mybir.dt.float16
    F32 = mybir.dt.float32
    MUL = mybir.AluOpType.mult
    ADD = mybir.AluOpType.add
    EXP = mybir.ActivationFunctionType.Exp

    nc = bacc.Bacc(trn_type="TRN2", target_bir_lowering=False, debug=False,
                   enable_asserts=False)
    xTp = nc.dram_tensor("xTp", [P, NQT, NE, QT], F16, kind="ExternalInput").ap()
    wq_t = nc.dram_tensor("wq_t", [P, NE, C], F16, kind="ExternalInput").ap()
    wk_t = nc.dram_tensor("wk_t", [P, NE, C], F16, kind="ExternalInput").ap()
    wv_t = nc.dram_tensor("wv_t", [P, NE, C], F16, kind="ExternalInput").ap()
    wo_p = nc.dram_tensor("wo_p", [P, 2, D], F16, kind="ExternalInput").ap()
    cos2 = nc.dram_tensor("cos2", [P, S], F16, kind="ExternalInput").ap()
    sin2 = nc.dram_tensor("sin2", [P, S], F16, kind="ExternalInput").ap()
    mask2 = nc.dram_tensor("mask2", [P, 2, P], F16, kind="ExternalInput").ap()
    onesd = nc.dram_tensor("onesd", [P, DH], F16, kind="ExternalInput").ap()
    y = nc.dram_tensor("y", [S, D], F16, kind="ExternalOutput").ap()

    with tile.TileContext(nc) as tc:
        with tc.tile_pool(name="keep", bufs=1) as keep, \
             tc.tile_pool(name="ph1", bufs=2) as ph1, \
             tc.tile_pool(name="swp", bufs=2) as swp, \
             tc.tile_pool(name="ptp", bufs=6) as ptp, \
             tc.tile_pool(name="normp", bufs=2) as normp, \
             tc.tile_pool(name="work", bufs=2) as work, \
             tc.tile_pool(name="psA", bufs=2, space="PSUM") as psA, \
             tc.tile_pool(name="psS", bufs=2, space="PSUM") as psS, \
             tc.tile_pool(name="psO", bufs=1, space="PSUM") as psO:

            # ---------------- persistent tiles ----------------
            qk_pair = {(w, pr): keep.tile([P, S], F16, tag=f"{w}{pr}",
                                          name=f"{w}{pr}")
                       for w in ("q", "k") for pr in range(2)}
            v_ext = keep.tile([P, NKT, HPC * (DH + 1)], F16, tag="vext")
            attnP = [keep.tile([P, S], F16, tag=f"attnP{pr}",
                               name=f"attnP{pr}") for pr in range(2)]
            wo_sb = keep.tile([P, 2, D], F16, tag="wo")
            cos_sb = keep.tile([P, S], F16, tag="cos")
            sin_sb = keep.tile([P, S], F16, tag="sin")
            wq_sb = keep.tile([P, NE, C], F16, tag="wq")
            wk_sb = keep.tile([P, NE, C], F16, tag="wk")
            wv_sb = keep.tile([P, NE, C], F16, tag="wv")
            mask_sb = keep.tile([P, 2, P], F16, tag="mask")
            ones_sb = keep.tile([DH + 1, DH], F16, tag="ones")

            vx = v_ext.rearrange("p t (h x) -> p t h x", h=HPC)

            # ---------------- input loads ----------------
            # phase1(0) blockers first: x chunk 0 and the q/k/v weights.
            xts = {0: ph1.tile([P, NE, QT], F16, tag="xt", name="xt0")}
            nc.sync.dma_start(xts[0][:, :4], xTp[:, 0, :4])
            nc.sync.dma_start(wq_sb[:, :4], wq_t[:, :4])
            nc.sync.dma_start(xts[0][:, 4:], xTp[:, 0, 4:])
            nc.sync.dma_start(wq_sb[:, 4:], wq_t[:, 4:])
            nc.sync.dma_start(wk_sb, wk_t)
            nc.sync.dma_start(wv_sb, wv_t)
            nc.sync.dma_start(cos_sb, cos2)
            nc.sync.dma_start(sin_sb, sin2)

            def late_loads():
                nc.sync.dma_start(mask_sb, mask2)
                nc.sync.dma_start(ones_sb, onesd[:DH + 1])
                nc.sync.dma_start(
                    vx[:, :, :, DH:],
                    onesd.rearrange("p (t h) -> p t h", t=NKT)[:, :, :, None])
                nc.sync.dma_start(wo_sb, wo_p)

            w_of = {"q": wq_sb, "k": wk_sb}

            def phase1(st):
                """QKV projections + RoPE for s-tile st."""
                if st in xts:
                    xt = xts.pop(st)
                else:
                    xt = ph1.tile([P, NE, QT], F16, tag="xt")
                    nc.sync.dma_start(xt, xTp[:, st])
                sl = slice(st * QT, (st + 1) * QT)
                for which in ("q", "k"):
                    for pr in range(2):
                        ps = psA.tile([P, QT], F32, tag="ps", name="ps")
                        for e in range(NE):
                            nc.tensor.matmul(
                                ps,
                                lhsT=w_of[which][:, e, pr * P:(pr + 1) * P],
                                rhs=xt[:, e],
                                start=(e == 0), stop=(e == NE - 1))
                        raw = qk_pair[(which, pr)][:, sl]
                        nc.vector.tensor_copy(raw, ps)
                        sw = swp.tile([P, QT], F16, tag="sw")
                        nc.vector.stream_shuffle(sw, raw, SHUF)
                        nc.vector.tensor_tensor(sw, sw, sin_sb[:, sl], MUL)
                        nc.vector.tensor_tensor(raw, raw, cos_sb[:, sl], MUL)
                        nc.vector.tensor_tensor(raw, raw, sw, ADD)
                for sb in range(4):
                    kt = st * 4 + sb
                    pv = psA.tile([P, QT], F32, tag="ps", name="pv")[:, :C]
                    for e in range(NE):
                        nc.tensor.matmul(
                            pv,
                            lhsT=xt[:, e, sb * P:(sb + 1) * P],
                            rhs=wv_sb[:, e],
                            start=(e == 0), stop=(e == NE - 1))
                    nc.vector.tensor_copy(
                        vx[:, kt, :, :DH],
                        pv.rearrange("p (h x) -> p h x", h=HPC))

            def attention(qt):
                nkt = 4 * qt + 4
                sl = slice(qt * QT, (qt + 1) * QT)
                for pr in range(2):
                    poA = psO.tile([DH + 1, QT], F32, tag="poA")
                    poB = psO.tile([DH + 1, QT], F32, tag="poB")
                    qh = [qk_pair[("q", pr)][hi * DH:(hi + 1) * DH]
                          for hi in range(2)]
                    kh = [qk_pair[("k", pr)][hi * DH:(hi + 1) * DH]
                          for hi in range(2)]
                    for kt in range(nkt):
                        j = kt - 4 * qt   # >= 0 on diagonal blocks
                        lo = max(j, 0) * P
                        ps = psS.tile([P, 2, QT], F32, tag="ps", name="pss")
                        for hi in range(2):
                            nc.tensor.matmul(
                                ps[:, hi, lo:],
                                lhsT=kh[hi][:, kt * P:(kt + 1) * P],
                                rhs=qh[hi][:, qt * QT + lo:(qt + 1) * QT])
                        pt = ptp.tile([P, 2, QT], F16, tag="pt")
                        nc.scalar.activation(pt[:, :, lo:], ps[:, :, lo:],
                                             EXP, scale=0.125)
                        if j >= 0:
                            nc.gpsimd.tensor_tensor(
                                pt[:, :, lo:lo + P], pt[:, :, lo:lo + P],
                                mask_sb, MUL)
                        nc.tensor.matmul(poA[:, lo:],
                                         lhsT=vx[:, kt, 2 * pr],
                                         rhs=pt[:, 0, lo:],
                                         start=(kt == 0), stop=(kt == nkt - 1))
                        nc.tensor.matmul(poB[:, lo:],
                                         lhsT=vx[:, kt, 2 * pr + 1],
                                         rhs=pt[:, 1, lo:],
                                         start=(kt == 0), stop=(kt == nkt - 1))
                    # free poA/poB (psO banks) as fast as possible: copy the
                    # attn rows out on ACT+DVE in parallel with the den recip,
                    # so the next pair's AV chain can start ~3us earlier.
                    den2 = normp.tile([DH + 1, 2, QT], F16, tag="den2")
                    pos = normp.tile([DH, 2, QT], F16, tag="pos")
                    nc.scalar.copy(pos[:, 0], poA[:DH])
                    nc.vector.tensor_copy(pos[:, 1], poB[:DH])
                    with nc.allow_low_precision(reason="softmax denom recip"):
                        nc.vector.reciprocal(den2[DH:, 0], poA[DH:])
                        nc.vector.reciprocal(den2[DH:, 1], poB[DH:])
                    bcA = psA.tile([P, QT], F32, tag="ps", name="bcA")[:DH]
                    bcB = psA.tile([P, QT], F32, tag="ps", name="bcB")[:DH]
                    nc.tensor.matmul(bcA, lhsT=ones_sb[DH:], rhs=den2[DH:, 0])
                    nc.tensor.matmul(bcB, lhsT=ones_sb[DH:], rhs=den2[DH:, 1])
                    atmp = swp.tile([DH, QT], F16, tag="atmp")
                    nc.vector.tensor_tensor(attnP[pr][:DH, sl],
                                            pos[:, 0], bcA, MUL)
                    nc.vector.tensor_tensor(atmp, pos[:, 1], bcB, MUL)
                    nc.sync.dma_start(attnP[pr][DH:, sl], atmp)

            def proj(qt):
                """Output projection for the 512 s-rows of q-tile qt."""
                ysb = work.tile([P, 4, D], F16, tag="ysb")
                for sb in range(4):
                    sc = qt * 4 + sb
                    for et in range(2):
                        psy = psA.tile([P, QT], F32, tag="ps", name="psy")
                        for pr in range(2):
                            nc.tensor.matmul(
                                psy,
                                lhsT=attnP[pr][:, sc * P:(sc + 1) * P],
                                rhs=wo_sb[:, pr, et * QT:(et + 1) * QT],
                                start=(pr == 0), stop=(pr == 1))
                        dst = ysb[:, sb, et * QT:(et + 1) * QT]
                        if et == 1 and sb % 2 == 1:
                            nc.scalar.copy(dst, psy)
                        else:
                            nc.vector.tensor_copy(dst, psy)
                nc.sync.dma_start(
                    y[qt * QT:(qt + 1) * QT].rearrange("(c p) e -> p c e",
                                                       p=P), ysb)

            phase1(0)
            late_loads()
            phase1(1)
            attention(0)
            proj(0)
            phase1(2)
            attention(1)
            proj(1)
            phase1(3)
            attention(2)
            proj(2)
            attention(3)
            proj(3)
    nc.compile()
    return nc


def _get_nc():
    if "nc" not in _cache:
        _cache["nc"] = _build()
    return _cache["nc"]


def _host_inputs(x, Wq, Wk, Wv, Wo, cos, sin):
    """Build the 8 per-core input dicts (fp16, prepacked layouts)."""
    f16 = np.float16
    ordH = (np.arange(HPC)[:, None] * DH + PERM64[None, :]).reshape(-1)  # [256]

    cosT = np.ascontiguousarray(cos.T).astype(np.float32)     # [DH, S]
    sinT = np.ascontiguousarray(sin.T).astype(np.float32)
    sinS = np.concatenate([-sinT[:DH // 2], sinT[DH // 2:]], axis=0)
    cos2 = np.tile(cosT[PERM64], (2, 1)).astype(f16)          # [128, S]
    sin2 = np.tile(sinS[PERM64], (2, 1)).astype(f16)
    mask1 = (np.arange(P)[:, None] <= np.arange(P)[None, :])
    mask2 = np.stack([mask1, mask1], axis=1).astype(f16)      # [128, 2, 128]
    onesd = np.ones((P, DH), f16)

    in_maps = []
    for c in range(NCORES):
        b, g = divmod(c, 4)
        cs = slice(C * g, C * g + C)
        xb = np.asarray(x[b], np.float32)                     # [S, D]
        xTp = xb.reshape(NQT, QT, NE, P).transpose(3, 0, 2, 1)
        wq_o = np.asarray(Wq, np.float32)[cs][ordH]           # [256, D]
        wk_o = np.asarray(Wk, np.float32)[cs][ordH]
        wv_o = np.asarray(Wv, np.float32)[cs]
        wo_o = np.asarray(Wo, np.float32).T[cs]               # [256, D]
        in_maps.append({
            "xTp": np.ascontiguousarray(xTp).astype(f16),
            "wq_t": np.ascontiguousarray(
                wq_o.T.reshape(NE, P, C).transpose(1, 0, 2)).astype(f16),
            "wk_t": np.ascontiguousarray(
                wk_o.T.reshape(NE, P, C).transpose(1, 0, 2)).astype(f16),
            "wv_t": np.ascontiguousarray(
                wv_o.T.reshape(NE, P, C).transpose(1, 0, 2)).astype(f16),
            "wo_p": np.ascontiguousarray(
                wo_o.reshape(2, P, D).transpose(1, 0, 2)).astype(f16),
            "cos2": cos2, "sin2": sin2, "mask2": mask2, "onesd": onesd,
        })
    return in_maps


def run(x, Wq, Wk, Wv, Wo, cos, sin, mask=None, trace=False, **trace_kw):
    import os
    import time
    if not trace:
        # The axon NTFF-profile hook is not installed in all containers;
        # make sure an inherited BASS_TRACE=1 can't send us down that path.
        os.environ.setdefault("BASS_NEVER_TRACE", "1")
    from concourse.bass_utils import run_bass_kernel_spmd
    nc = _get_nc()
    in_maps = _host_inputs(np.asarray(x), np.asarray(Wq), np.asarray(Wk),
                           np.asarray(Wv), np.asarray(Wo), np.asarray(cos),
                           np.asarray(sin))
    try:
        res = run_bass_kernel_spmd(nc, in_maps, core_ids=list(range(NCORES)),
                                   trace=trace, **trace_kw)
    except Exception:
        # one retry for transient device states (e.g. NRT_EXEC_UNIT errors)
        time.sleep(15)
        res = run_bass_kernel_spmd(nc, in_maps, core_ids=list(range(NCORES)),
                                   trace=trace, **trace_kw)
    parts = [r["y"].astype(np.float32) for r in res.results]
    out = np.stack([parts[0] + parts[1] + parts[2] + parts[3],
                    parts[4] + parts[5] + parts[6] + parts[7]])
    return out.astype(np.float32), res


def kernel(x, Wq, Wk, Wv, Wo, cos, sin, mask=None, **_):
    out, _res = run(x, Wq, Wk, Wv, Wo, cos, sin, mask)
    return out
